# revision 50
# baseline (speedup 1.0000x reference)
"""Trainium2 Bass kernel for multi-head attention (B=4, N=2048, C=768, H=12).

Sharding: 8 cores = 4 batches x 2 head-halves. Each core computes Q/K/V and
attention for its 6 heads (3 head-pairs) over the full 2048-token sequence,
then the final projection restricted to its 384 feature columns, producing a
partial [2048, 768] output (f16). The host sums the two partials per batch
(the even core folds in the bias). No duplicated projection work, no
collectives.

All matmul operands are fp16, accumulation f32 in PSUM, softmax norm in f32.

The kernel is paced by ScalarE exp (~25M exps/core, [128,1024] per ACTIVATE =
~1147ns each). Everything else hides under the exp stream:
- startup: inputs stream on the gpsimd DMA queue in need-order (pair-0 K/Q
  weights, x chunk 0, ...); dummy matmuls on constants warm the PE HAM clock
  while the first DMAs land, so the first real matmuls run at 2.4 GHz.
- blocks interleave pairs (0,*),(1,0),(2,0),(1,1),(2,1),... so the output
  projection (which needs all three pairs' outT for a token range) spreads
  over two blocks of PE slack instead of piling up at the end.
- scores are software-pipelined one key-tile ahead of the exp that consumes
  them; AV accumulates v'=[v|ones] so row 64 carries the softmax denominator.
- normalization: 1/den via reciprocal_approx_fast (5x faster than the exact
  DVE op), broadcast across partitions by a PE outer product into a
  short-lived apsum slot (sharing the po banks with the live accumulators
  would push every fin a full block late; gpsimd.partition_broadcast gives
  wrong results on hardware).
"""

import numpy as np

B, N, C = 4, 2048, 768
H, HD = 12, 64
SCALE = HD ** -0.5
P = 128
CT = C // P          # 6 contraction tiles for QKV projections
HC = C // 2          # 384 feature columns per core
PCT = HC // P        # 3 contraction tiles for the final projection
PAIRS = 3            # head pairs per core
JT = N // P          # 16 key tiles
IB = N // 512        # 4 query blocks
TKB = 512            # token-block width of projection matmuls
NCORES = 8

_cache = {}


def _build_bass():
    import concourse.bass as bass
    import concourse.tile as tile
    import concourse.mybir as mybir
    from concourse import bacc
    from concourse.bass import ts, ds
    from contextlib import ExitStack

    f32 = mybir.dt.float32
    f16 = mybir.dt.float16
    Exp = mybir.ActivationFunctionType.Exp

    nc = bacc.Bacc("TRN2", target_bir_lowering=False, debug=False)

    # all inputs pre-swizzled on the host to partition-major layouts so
    # every load is a fully contiguous DMA; wq/wk additionally pair-major
    # so pair 0 can load first
    xt_d = nc.dram_tensor("xt", [P, N * CT], f16, kind="ExternalInput").ap()
    wq_d = nc.dram_tensor("wq", [P, PAIRS * CT * P], f16, kind="ExternalInput").ap()
    wk_d = nc.dram_tensor("wk", [P, PAIRS * CT * P], f16, kind="ExternalInput").ap()
    wv_d = nc.dram_tensor("wv", [P, CT * HC], f16, kind="ExternalInput").ap()
    wp_d = nc.dram_tensor("wp", [P, PCT * C], f16, kind="ExternalInput").ap()
    bb_d = nc.dram_tensor("bb", [P, C], f32, kind="ExternalInput").ap()
    sel_d = nc.dram_tensor("sel", [2, P], f16, kind="ExternalInput").ap()
    out_d = nc.dram_tensor("out", [N, C], f16, kind="ExternalOutput").ap()

    xt_r = xt_d.rearrange("p (t o n) -> p t o n", t=N // TKB, o=CT)
    wq_r = wq_d.rearrange("p (h o n) -> p h o n", h=PAIRS, o=CT)
    wk_r = wk_d.rearrange("p (h o n) -> p h o n", h=PAIRS, o=CT)
    wv_r = wv_d.rearrange("p (o n) -> p o n", o=CT)
    wp_r = wp_d.rearrange("p (o n) -> p o n", o=PCT)
    out_r = out_d.rearrange("(t p) n -> t p n", p=P)

    with tile.TileContext(nc) as tc:
        with ExitStack() as ctx:
            persist = ctx.enter_context(tc.tile_pool(name="persist", bufs=1))
            xt_sb = persist.tile([P, N // TKB, CT, TKB], f16, name="xt_sb")
            wk_sb = persist.tile([P, PAIRS, CT, P], f16, name="wk_sb")
            wq_sb = persist.tile([P, PAIRS, CT, P], f16, name="wq_sb")
            wv_sb = persist.tile([P, CT, HC], f16, name="wv_sb")
            wp_sb = persist.tile([P, PCT, C], f16, name="wp_sb")
            bias_sb = persist.tile([P, C], f32, name="bias_sb")

            # input DMAs in need-order on the gpsimd queue (that engine's
            # preamble finishes ~1us before sync's, so first bytes land
            # earlier); den/out DMAs use the sync queue instead.
            nc.gpsimd.dma_start(wk_sb[:, 0], wk_r[:, 0])
            nc.gpsimd.dma_start(wq_sb[:, 0], wq_r[:, 0])
            # x chunk 0 split in two so the first kq matmuls start earlier
            nc.gpsimd.dma_start(xt_sb[:, 0, 0:3], xt_r[:, 0, 0:3])
            nc.gpsimd.dma_start(xt_sb[:, 0, 3:6], xt_r[:, 0, 3:6])
            nc.gpsimd.dma_start(wv_sb[:], wv_r)
            nc.gpsimd.dma_start(xt_sb[:, 1], xt_r[:, 1])
            nc.gpsimd.dma_start(xt_sb[:, 2], xt_r[:, 2])
            nc.gpsimd.dma_start(wk_sb[:, 1:3], wk_r[:, 1:3])
            nc.gpsimd.dma_start(xt_sb[:, 3], xt_r[:, 3])
            nc.gpsimd.dma_start(wq_sb[:, 1:3], wq_r[:, 1:3])
            nc.gpsimd.dma_start(wp_sb[:], wp_r)
            nc.gpsimd.dma_start(bias_sb[:], bb_d)
            sel2 = persist.tile([2, P], f16, name="sel2")
            nc.gpsimd.dma_start(sel2[:], sel_d)

            # pair-packed K/Q: partitions 0:64 even head, 64:128 odd head
            kT_sb = persist.tile([P, PAIRS, N], f16, name="kT_sb")
            qT_sb = persist.tile([P, PAIRS, N], f16, name="qT_sb")
            # V + ones column: [keys 128, key-tile, head, 66] (col 64 = ones)
            v_all = persist.tile([P, JT, 6, 66], f16, name="v_all")
            outT_sb = persist.tile([P, PAIRS, N], f16, name="outT_sb")
            ones_sb = persist.tile([33, 64], f16, name="ones_sb")
            warm_c = persist.tile([P, TKB], f16, name="warm_c")
            # rows 0/32 receive each pair's two denominators; rows 1-31 only
            # feed wasted reciprocal lanes but must be nonzero and initialized
            den_q = persist.tile([2, 512], f32, name="den_q")
            with nc.allow_low_precision(reason="ones constant is exact in f16"):
                nc.vector.tensor_copy(
                    v_all[:, :, :, 64:66],
                    nc.const_aps.tensor(1.0, [P, JT, 6, 2], f32),
                )
                nc.vector.tensor_copy(
                    ones_sb[:], nc.const_aps.tensor(1.0, [33, 64], f32)
                )
                nc.vector.tensor_copy(
                    warm_c[:], nc.const_aps.tensor(1.0, [P, TKB], f32)
                )
            nc.vector.tensor_copy(
                den_q[:], nc.const_aps.tensor(1.0, [2, 512], f32)
            )
            # f16 bias row for the tail's bias-fold matmul
            bias16 = persist.tile([1, C], f16, name="bias16")
            with nc.allow_low_precision(reason="f16 bias row"):
                nc.vector.tensor_copy(bias16[:], bias_sb[0:1, :])
            # dummy exp forces the ACT table load off the critical path
            warm_sb = persist.tile([1, 64], f16, name="warm_sb")
            nc.scalar.activation(warm_sb[:], ones_sb[0:1, :], Exp)

            apsum = ctx.enter_context(
                tc.tile_pool(name="apsum", bufs=2, space="PSUM")
            )
            spsum = ctx.enter_context(
                tc.tile_pool(name="spsum", bufs=2, space="PSUM")
            )
            opsum = ctx.enter_context(
                tc.tile_pool(name="opsum", bufs=2, space="PSUM")
            )
            expt_pool = ctx.enter_context(tc.tile_pool(name="expt", bufs=6))
            nrm_pool = ctx.enter_context(tc.tile_pool(name="nrm", bufs=2))
            poS_pool = ctx.enter_context(tc.tile_pool(name="poSp", bufs=4))
            outsb_pool = ctx.enter_context(tc.tile_pool(name="outsb", bufs=6))

            # ~5us of dummy matmuls on constants while input DMAs stream:
            # keeps the PE HAM window busy so real matmuls start at 2.4 GHz.
            # Must be full-array matmuls: a near-empty stationary doesn't
            # register as PE activity for the HAM clock gate.
            warm_ps = apsum.tile([P, TKB], f32, tag="aps", name="warm_ps")
            for _ in range(12):
                nc.tensor.matmul(
                    warm_ps[:],
                    warm_c[:, 0:P],
                    warm_c[:],
                    start=True,
                    stop=True,
                )

            def kq_group(p, tb, which):
                w_sb, dst = (
                    (wk_sb, kT_sb) if which == "k" else (wq_sb, qT_sb)
                )
                ps = apsum.tile([P, TKB], f32, tag="aps")
                for c in range(CT):
                    nc.tensor.matmul(
                        ps[:],
                        w_sb[:, p, c, :],
                        xt_sb[:, tb, c, :],
                        start=(c == 0),
                        stop=(c == CT - 1),
                    )
                with nc.allow_low_precision(reason="f16 kq path"):
                    nc.vector.tensor_copy(dst[:, p, ts(tb, TKB)], ps[:])

            def kq_pair(s0, s1):
                # two kq groups with interleaved accumulation chains, like
                # v_pair: consecutive matmuls alternate PSUM banks so the
                # per-chain drain serialization disappears
                specs = [s0, s1]

                def th():
                    pss = [
                        apsum.tile([P, TKB], f32, tag="aps", name="kqp")
                        for _ in specs
                    ]
                    for c in range(CT):
                        for ps, (p, tb, w) in zip(pss, specs):
                            w_sb = wk_sb if w == "k" else wq_sb
                            nc.tensor.matmul(
                                ps[:], w_sb[:, p, c, :], xt_sb[:, tb, c, :],
                                start=(c == 0), stop=(c == CT - 1),
                            )
                    with nc.allow_low_precision(reason="f16 kq path"):
                        for ps, (p, tb, w) in zip(pss, specs):
                            dst = kT_sb if w == "k" else qT_sb
                            nc.vector.tensor_copy(
                                dst[:, p, ts(tb, TKB)], ps[:]
                            )
                return th

            def kq_pieces(p, tb, which):
                # one kq group as six single-matmul thunks (~0.23us each) so
                # a background slot never exceeds the per-jt exp budget
                w_sb, dst = (
                    (wk_sb, kT_sb) if which == "k" else (wq_sb, qT_sb)
                )
                box = []

                def piece(c):
                    def th():
                        if c == 0:
                            box.append(
                                apsum.tile([P, TKB], f32, tag="aps", name="ps")
                            )
                        ps = box[0]
                        nc.tensor.matmul(
                            ps[:], w_sb[:, p, c, :], xt_sb[:, tb, c, :],
                            start=(c == 0), stop=(c == CT - 1),
                        )
                        if c == CT - 1:
                            with nc.allow_low_precision(reason="f16 kq path"):
                                nc.vector.tensor_copy(
                                    dst[:, p, ts(tb, TKB)], ps[:]
                                )
                    return th

                return [piece(c) for c in range(CT)]

            def v_pair(t0, t1=None):
                # V projection for one or two 128-token tiles, all 6 heads:
                # token-tile stationary, wv moving. Two tiles interleave
                # their accumulation chains so consecutive matmuls hit
                # different PSUM banks and the drain serialization of a
                # single chain disappears.
                tts = [t0] if t1 is None else [t0, t1]
                pss = [apsum.tile([P, TKB], f32, tag="aps", name="vps") for _ in tts]
                for c in range(CT):
                    for ps, tt in zip(pss, tts):
                        nc.tensor.matmul(
                            ps[:, 0:HC],
                            xt_sb[:, tt // 4, c, ts(tt % 4, P)],
                            wv_sb[:, c, :],
                            start=(c == 0),
                            stop=(c == CT - 1),
                        )
                with nc.allow_low_precision(reason="f16 value path"):
                    for ps, tt in zip(pss, tts):
                        nc.vector.tensor_copy(
                            v_all[:, tt, :, 0:64],
                            ps[:, 0:HC].rearrange("p (h e) -> p h e", e=64),
                        )

            def out_chunk(git, ci, ob, pp=None, skip_mm=False, split_dma=False):
                # half of the final projection for one 128-token tile
                n0, n1 = ((0, 512), (512, 768))[ci]
                if pp is None:
                    pp = apsum.tile([P, TKB], f32, tag="aps")
                if not skip_mm:
                    for t in range(PAIRS):
                        nc.tensor.matmul(
                            pp[:, 0 : n1 - n0],
                            outT_sb[:, t, ds(git * P, P)],
                            wp_sb[:, t, n0:n1],
                            start=(t == 0),
                            stop=(t == PAIRS - 1),
                        )
                with nc.allow_low_precision(reason="f16 partial output"):
                    nc.vector.tensor_add(
                        ob[:, n0:n1], pp[:, 0 : n1 - n0], bias_sb[:, n0:n1]
                    )
                if split_dma:
                    # tail: ship each half as soon as its bias add lands
                    nc.sync.dma_start(out_r[git][:, n0:n1], ob[:, n0:n1])
                elif ci == 1:
                    nc.sync.dma_start(out_r[git], ob[:])

            def og_thunks(gits):
                th = []
                for g in gits:
                    ob_box = []

                    def c0(g=g, ob_box=ob_box):
                        ob_box.append(
                            outsb_pool.tile([P, C], f16, tag="ob", name="ob")
                        )
                        out_chunk(g, 0, ob_box[0])

                    def c1(g=g, ob_box=ob_box):
                        out_chunk(g, 1, ob_box[0])

                    th += [c0, c1]
                return th

            def attention(p, ib, with_v=False, bg=None, fin=None, last=False):
                po0 = opsum.tile([P, 512], f32, tag="po")
                po1 = opsum.tile([P, 512], f32, tag="po")
                pos = (po0, po1)

                def scores(jt):
                    ss = spsum.tile([P, 1024], f32, tag="ss")
                    nc.tensor.matmul(
                        ss[:, 0:512],
                        kT_sb[0:64, p, ts(jt, P)],
                        qT_sb[0:64, p, ts(ib, 512)],
                        start=True,
                        stop=True,
                    )
                    nc.tensor.matmul(
                        ss[:, 512:1024],
                        kT_sb[64:128, p, ts(jt, P)],
                        qT_sb[64:128, p, ts(ib, 512)],
                        start=True,
                        stop=True,
                    )
                    return ss

                ss_cur = scores(0)
                if with_v:
                    v_pair(0, 1)
                for jt in range(JT):
                    et = expt_pool.tile([P, 1024], f16, tag="et")
                    nc.scalar.activation(et[:], ss_cur[:], Exp, scale=SCALE)
                    # next key-tile's scores run on the PE while the exp
                    # above works through this one
                    if jt < JT - 1:
                        ss_cur = scores(jt + 1)
                        if with_v:
                            if jt % 2 == 0 and jt < 14:
                                v_pair(jt + 2, jt + 3)
                    if bg is not None and jt < len(bg) and bg[jt] is not None:
                        bg[jt]()
                    if jt == 7 and fin is not None:
                        fin()
                    for hh in range(2):
                        nc.tensor.matmul(
                            pos[hh][0:65, :],
                            v_all[:, jt, 2 * p + hh, 0:65],
                            et[:, hh * 512 : (hh + 1) * 512],
                            start=(jt == 0),
                            stop=(jt == JT - 1),
                        )
                poS_all = []
                for hh in range(2):
                    poS = poS_pool.tile([65, 512], f32, tag="poS")
                    nc.vector.tensor_copy(poS[:], pos[hh][0:65, :])
                    # stack this head's denominator at partition 32*hh
                    nc.sync.dma_start(
                        den_q[hh : hh + 1, :], poS[64:65, :]
                    )
                    poS_all.append(poS)
                rd32 = nrm_pool.tile([2, 512], f32, tag="rd32")
                nc.vector.reciprocal_approx_fast(rd32[:], den_q[:])
                rd_q = nrm_pool.tile([2, 512], f16, tag="rd_q")
                with nc.allow_low_precision(reason="f16 reciprocal scale"):
                    nc.vector.tensor_copy(rd_q[:], rd32[:])

                def fin_thunk():
                    # fused partition-broadcast of both heads' 1/den in one
                    # PE outer product: out[p,q] = sum_r sel2[r,p]*rd[r,q]
                    # = rd_h0[q] on partitions 0:64, rd_h1[q] on 64:128.
                    # apsum mid-stream; po banks for the final block (the
                    # tail prefill occupies all aps/ss slots with work
                    # depending on this output).
                    rb_ps = (opsum if last else apsum).tile(
                        [P, 512], f32, tag="po" if last else "aps"
                    )
                    nc.tensor.matmul(
                        rb_ps[:], sel2[:], rd_q[:], start=True, stop=True
                    )
                    rbs = [rb_ps[0:64, :], rb_ps[64:128, :]]
                    with nc.allow_low_precision(reason="f16 attn output"):
                        if last:
                            # git-12 muls first so the tail projections can
                            # start while the remaining tokens normalize
                            for lo, n in ((0, P), (P, 3 * P)):
                                for hh in range(2):
                                    nc.vector.tensor_mul(
                                        outT_sb[
                                            hh * 64 : (hh + 1) * 64,
                                            p,
                                            ds(ib * 512 + lo, n),
                                        ],
                                        poS_all[hh][0:64, ds(lo, n)],
                                        rbs[hh][:, ds(lo, n)],
                                    )
                        else:
                            for hh in range(2):
                                nc.vector.tensor_mul(
                                    outT_sb[hh * 64 : (hh + 1) * 64, p, ts(ib, 512)],
                                    poS_all[hh][0:64, :],
                                    rbs[hh][:, :],
                                )

                return fin_thunk

            def slots(d, njt=JT):
                out = [None] * njt
                for k, th in d.items():
                    if isinstance(th, list):
                        def chain(ths=th):
                            for t in ths:
                                t()
                        out[k] = chain
                    else:
                        out[k] = th
                return out

            og = og_thunks(range(12))   # 24 chunk thunks for gits 0..11

            # first K and Q projections with interleaved accumulation chains:
            # the c0-2 matmuls of both run as soon as the first half of x
            # chunk 0 lands, the rest right after the second half's semaphore
            psk = apsum.tile([P, TKB], f32, tag="aps", name="psk")
            psq = apsum.tile([P, TKB], f32, tag="aps", name="psq")
            for c in range(CT):
                nc.tensor.matmul(
                    psk[:], wk_sb[:, 0, c, :], xt_sb[:, 0, c, :],
                    start=(c == 0), stop=(c == CT - 1),
                )
                nc.tensor.matmul(
                    psq[:], wq_sb[:, 0, c, :], xt_sb[:, 0, c, :],
                    start=(c == 0), stop=(c == CT - 1),
                )
            with nc.allow_low_precision(reason="f16 kq path"):
                nc.vector.tensor_copy(kT_sb[:, 0, ts(0, TKB)], psk[:])
                nc.vector.tensor_copy(qT_sb[:, 0, ts(0, TKB)], psq[:])
            # each pair's first block carries that pair's remaining K
            # projections (scores consume kT tile-by-tile); q projections
            # land one block before their query block needs them
            fin = attention(0, 0, with_v=True, bg=slots({
                2: kq_pair((0, 1, "k"), (0, 2, "k")),
                8: kq_pair((0, 3, "k"), (0, 1, "q")),
            }))
            def hgroup(p, tb, w, s, d):
                # six pieces over four slots: 2,1,2,1 — keeps every slot's
                # PE load under the exp budget so the stream never slips.
                # Groups starting at slot 12 compress to 2,2,2 so the closing
                # copy lands before the end-of-block DVE rush (poS casts).
                pc = kq_pieces(p, tb, w)
                if s >= 12:
                    d.setdefault(s, []).extend(pc[0:2])
                    d.setdefault(s + 1, []).extend(pc[2:4])
                    d.setdefault(s + 2, []).extend(pc[4:6])
                else:
                    d.setdefault(s, []).extend(pc[0:2])
                    d.setdefault(s + 1, []).append(pc[2])
                    d.setdefault(s + 2, []).extend(pc[3:5])
                    d.setdefault(s + 3, []).append(pc[5])

            def hsingles(p, tb, w, s, d):
                # six pieces, one per slot, for blocks with og work
                for k, pc in enumerate(kq_pieces(p, tb, w)):
                    d.setdefault(s + k, []).append(pc)

            d = {}
            hgroup(0, 2, "q", 0, d); hgroup(1, 0, "k", 4, d)
            hgroup(1, 0, "q", 8, d); hgroup(1, 1, "k", 12, d)
            fin = attention(0, 1, bg=slots(d), fin=fin)
            d = {}
            hgroup(0, 3, "q", 0, d); hgroup(1, 2, "k", 5, d)
            hgroup(1, 3, "k", 10, d)
            fin = attention(0, 2, bg=slots(d), fin=fin)
            d = {}
            hgroup(2, 0, "k", 0, d); hgroup(2, 0, "q", 5, d)
            hgroup(2, 1, "k", 10, d)
            fin = attention(0, 3, bg=slots(d), fin=fin)
            d = {}
            hgroup(2, 2, "k", 0, d); hgroup(2, 3, "k", 4, d)
            hgroup(1, 1, "q", 12, d)
            fin = attention(1, 0, bg=slots(d), fin=fin)
            d = {}
            hgroup(2, 2, "q", 2, d); hgroup(2, 3, "q", 8, d)
            hgroup(1, 2, "q", 12, d)
            fin = attention(2, 0, bg=slots(d), fin=fin)
            # og(gits of ib) needs outT of all three pairs for that ib: the
            # last fin to land is fin(2, ib), fired at jt7 of the following
            # block, so each og batch spreads over the next two blocks.
            # Late q projections ride along in the og blocks' spare slots.
            d = {8: og[0], 9: og[1], 11: og[2], 12: og[3]}
            hsingles(2, 1, "q", 0, d)
            fin = attention(1, 1, bg=slots(d), fin=fin)
            d = {0: og[4], 2: og[5], 4: og[6], 6: og[7]}
            hsingles(1, 3, "q", 8, d)
            fin = attention(2, 1, bg=slots(d), fin=fin)
            fin = attention(1, 2, bg=slots({
                8: og[8], 9: og[9], 11: og[10], 12: og[11],
            }), fin=fin)
            fin = attention(2, 2, bg=slots({
                0: og[12], 2: og[13], 4: og[14], 6: og[15],
            }), fin=fin)
            fin = attention(1, 3, bg=slots({
                8: og[16], 9: og[17], 11: og[18], 12: og[19],
            }), fin=fin)
            fin = attention(2, 3, bg=slots({
                0: og[20], 2: og[21], 4: og[22], 6: og[23],
            }), fin=fin, last=True)

            # tail: prefill the pair-0/1 contributions for gits 12-14 while
            # the last exps and the final normalization chain run, then
            # finish with the pair-2 matmuls, bias adds and output DMAs.
            sst0 = spsum.tile([P, 1024], f32, tag="ss", name="sst0")
            sst1 = spsum.tile([P, 1024], f32, tag="ss", name="sst1")
            tail_slots = [
                apsum.tile([P, TKB], f32, tag="aps", name="tp0"),
                apsum.tile([P, TKB], f32, tag="aps", name="tp1"),
                sst0[:, 0:512],
                sst0[:, 512:1024],
                sst1[:, 0:512],
                sst1[:, 512:1024],
            ]
            tail_gc = [(g, ci) for g in range(12, 15) for ci in (0, 1)]
            for k, (g, ci) in enumerate(tail_gc):
                n0, n1 = ((0, 512), (512, 768))[ci]
                pp = tail_slots[k]
                for t in (0, 1):
                    nc.tensor.matmul(
                        pp[:, 0 : n1 - n0],
                        outT_sb[:, t, ds(g * P, P)],
                        wp_sb[:, t, n0:n1],
                        start=(t == 0),
                        stop=False,
                    )
                # fold the bias in as a rank-1 matmul so the psum->SBUF move
                # below is a pure copy that the idle Scalar engine can do
                nc.tensor.matmul(
                    pp[:, 0 : n1 - n0],
                    warm_c[0:1, 0:P],
                    bias16[0:1, n0:n1],
                    start=False,
                    stop=False,
                )
            # keep the PE warm through the final reciprocal chain (a >3.4us
            # idle would re-throttle the HAM clock and halve the speed of
            # the remaining projection matmuls); the po banks are free once
            # the last poS casts complete
            warm_tl = opsum.tile([P, 512], f32, tag="po", name="warm_tl")
            for _ in range(6):
                nc.tensor.matmul(
                    warm_tl[:], warm_c[:, 0:P], warm_c[:], start=True, stop=True
                )
            fin()                                   # outT(2, 3)
            obs = [
                outsb_pool.tile([P, C], f16, tag="ob", name=f"obt{g}")
                for g in range(4)
            ]
            for k, (g, ci) in enumerate(tail_gc):
                n0, n1 = ((0, 512), (512, 768))[ci]
                pp = tail_slots[k]
                nc.tensor.matmul(
                    pp[:, 0 : n1 - n0],
                    outT_sb[:, 2, ds(g * P, P)],
                    wp_sb[:, 2, n0:n1],
                    start=False,
                    stop=True,
                )
                ob = obs[g - 12]
                with nc.allow_low_precision(reason="f16 partial output"):
                    nc.scalar.copy(ob[:, n0:n1], pp[:, 0 : n1 - n0])
                nc.sync.dma_start(out_r[g][:, n0:n1], ob[:, n0:n1])
            # git 15: full projection at the very end (bias folded, ACT copy)
            for ci in (0, 1):
                n0, n1 = ((0, 512), (512, 768))[ci]
                pp = apsum.tile([P, TKB], f32, tag="aps", name="pg15")
                for t in range(PAIRS):
                    nc.tensor.matmul(
                        pp[:, 0 : n1 - n0],
                        outT_sb[:, t, ds(15 * P, P)],
                        wp_sb[:, t, n0:n1],
                        start=(t == 0),
                        stop=False,
                    )
                nc.tensor.matmul(
                    pp[:, 0 : n1 - n0],
                    warm_c[0:1, 0:P],
                    bias16[0:1, n0:n1],
                    start=False,
                    stop=True,
                )
                with nc.allow_low_precision(reason="f16 partial output"):
                    nc.scalar.copy(obs[3][:, n0:n1], pp[:, 0 : n1 - n0])
                nc.sync.dma_start(out_r[15][:, n0:n1], obs[3][:, n0:n1])

    nc.compile()
    return nc


def _get_nc():
    if "nc" not in _cache:
        _cache["nc"] = _build_bass()
    return _cache["nc"]


def _prep_in_maps(x, w_qkv, w_proj, b_proj):
    x = np.asarray(x, np.float32)
    w_qkv = np.asarray(w_qkv, np.float32)
    w_proj = np.asarray(w_proj, np.float32)
    b_proj = np.asarray(b_proj, np.float32)

    def swz(w):
        # [C_in, M] -> partition-major [128, CT_in * M] (contiguous DMA)
        ct, m = w.shape[0] // P, w.shape[1]
        return np.ascontiguousarray(
            w.reshape(ct, P, m).transpose(1, 0, 2).reshape(P, ct * m)
        ).astype(np.float16)

    def swz_kq(w):
        # [C_in=768, 384] -> pair-major [128, PAIRS * CT * 128]
        return np.ascontiguousarray(
            w.reshape(CT, P, PAIRS, P).transpose(1, 2, 0, 3).reshape(P, -1)
        ).astype(np.float16)

    wq = np.ascontiguousarray(w_qkv[0:C].T)
    wk = np.ascontiguousarray(w_qkv[C : 2 * C].T)
    wv = np.ascontiguousarray(w_qkv[2 * C : 3 * C].T)
    wp = w_proj.T
    bb = np.ascontiguousarray(np.broadcast_to(b_proj[None, :], (P, C)))
    zb = np.zeros((P, C), np.float32)
    # 2-row selector for the fused 1/den broadcast
    sel = np.zeros((2, P), np.float16)
    sel[0, 0:64] = 1.0
    sel[1, 64:128] = 1.0

    in_maps = []
    for core in range(NCORES):
        b, half = core // 2, core % 2
        # x[b].T [C, N] -> [128, TB, CT, 512] partition-major, contiguous
        xt = np.ascontiguousarray(
            x[b].T.reshape(CT, P, N // TKB, TKB)
            .transpose(1, 2, 0, 3)
            .reshape(P, -1)
        ).astype(np.float16)
        sl = slice(half * HC, (half + 1) * HC)
        in_maps.append(
            {
                "xt": xt,
                "wq": swz_kq(wq[:, sl]),
                "wk": swz_kq(wk[:, sl]),
                "wv": swz(wv[:, sl]),
                "wp": swz(wp[sl, :]),
                "bb": bb if half == 0 else zb,
                "sel": sel,
            }
        )
    return in_maps


def run(x, w_qkv, w_proj, b_proj, trace=False):
    from concourse import bass_utils

    nc = _get_nc()
    in_maps = _prep_in_maps(x, w_qkv, w_proj, b_proj)
    br = bass_utils.run_bass_kernel_spmd(
        nc, in_maps, core_ids=list(range(NCORES)), trace=trace
    )
    y = np.empty((B, N, C), np.float32)
    for b in range(B):
        y[b] = np.asarray(br.results[2 * b]["out"], np.float32)
        y[b] += np.asarray(br.results[2 * b + 1]["out"], np.float32)
    return y, br


def kernel(x, w_qkv, w_proj, b_proj):
    y, _ = run(x, w_qkv, w_proj, b_proj, trace=False)
    return y
